# revision 1
# baseline (speedup 1.0000x reference)
"""Causal multi-head attention block (B=4,S=2048,D=1024,H=16) on 8 trn2 cores.

Sharding: data-parallel over batch (4) x tensor-parallel over head-groups (2).
Core c handles batch c//2, heads [8*(c%2), 8*(c%2)+8). Each core computes its
partial output projection; the host sums the two head-group partials per batch.
"""

import sys

for p in ("/opt/trn_rl_repo", "/root/.axon_site/_ro/trn_rl_repo"):
    if p not in sys.path:
        sys.path.insert(0, p)

import numpy as np
import ml_dtypes

import concourse.bass as bass
import concourse.mybir as mybir
import concourse.tile as tile
from concourse import bacc
from concourse.bass_utils import run_bass_kernel_spmd

FP32 = mybir.dt.float32
BF16 = mybir.dt.bfloat16
AF = mybir.ActivationFunctionType

B, S, D, H = 4, 2048, 1024, 16
DH = 64
N_CORES = 8
HPC = H // 2  # heads per core (head-group tensor parallel = 2)


def build_nc(s=S, d=D, hpc=HPC, dbg=False, reps=1, ablate=()):
    """Build the per-core SPMD program. All 8 cores run this same program."""
    P = 128
    KC = d // P              # feature chunks (contraction for qkv proj)
    NPAIR = hpc // 2         # head pairs
    VC = hpc * DH            # v columns / a columns per core
    QTS = 512                # query tile size
    NQT = s // QTS           # query tiles
    PC = VC // P             # proj contraction chunks (= NPAIR)
    NOUT = d // 512          # out-proj n tiles

    nc = bacc.Bacc("TRN2", target_bir_lowering=False, debug=False,
                   num_devices=N_CORES)

    xb = nc.dram_tensor("xb", [s, d], BF16, kind="ExternalInput")
    wqk = nc.dram_tensor("wqk", [d, 2 * VC], BF16, kind="ExternalInput")
    wv = nc.dram_tensor("wv", [d, VC], BF16, kind="ExternalInput")
    wp = nc.dram_tensor("wp", [VC, d], BF16, kind="ExternalInput")
    bqk = nc.dram_tensor("bqk", [2 * VC], FP32, kind="ExternalInput")
    bv = nc.dram_tensor("bv", [VC], FP32, kind="ExternalInput")
    bph = nc.dram_tensor("bph", [d], FP32, kind="ExternalInput")
    yp = nc.dram_tensor("yp", [s, d], FP32, kind="ExternalOutput")

    rsd = nc.dram_tensor("rsd", [hpc, 1024], FP32)  # recip-sum staging

    if dbg:
        qTo = nc.dram_tensor("qTo", [128, hpc // 2, s], FP32, kind="ExternalOutput")
        kTo = nc.dram_tensor("kTo", [128, hpc // 2, s], FP32, kind="ExternalOutput")
        vo = nc.dram_tensor("vo", [128, s // 128, hpc, DH + 1], FP32, kind="ExternalOutput")
        aTno = nc.dram_tensor("aTno", [128, hpc // 2, s], FP32, kind="ExternalOutput")

    def m_d(dd):
        k = np.arange(P)[:, None]
        q = np.arange(512)[None, :]
        return (k + dd <= q).astype(ml_dtypes.bfloat16)
    gm0_np = np.concatenate([m_d(0), m_d(128), m_d(0), m_d(128)], axis=1)
    gm1_np = np.concatenate([m_d(256), m_d(384), m_d(256), m_d(384)], axis=1)
    gm0_dram = nc.inline_tensor(gm0_np, name="gm0")
    gm1_dram = nc.inline_tensor(gm1_np, name="gm1")

    with tile.TileContext(nc) as tc:
        with (
            tc.tile_pool(name="singles", bufs=1) as singles,
            tc.tile_pool(name="xt", bufs=12) as xt_pool,
            tc.tile_pool(name="probs", bufs=2) as probs_pool,
            tc.tile_pool(name="norm", bufs=4) as norm_pool,
            tc.tile_pool(name="ysb", bufs=2) as y_pool,
            tc.tile_pool(name="mm512", bufs=2, space="PSUM") as mm_ps,
            tc.tile_pool(name="scps", bufs=1, space="PSUM") as sc_ps,
            tc.tile_pool(name="atps", bufs=1, space="PSUM") as at_ps,
        ):
            # ---- persistent SBUF state ----
            wqk_sb = singles.tile([P, KC, 2 * VC], BF16)
            wv_sb = singles.tile([P, KC, VC], BF16)
            wp_sb = singles.tile([P, PC, d], BF16)
            bqk_sb = singles.tile([P, 2 * VC // P], FP32)
            bv_rep = singles.tile([P, VC], FP32)
            bp_rep = singles.tile([P, d], FP32)
            gm0_sb = singles.tile([P, 2048], BF16)
            gm1_sb = singles.tile([P, 2048], BF16)
            qT = singles.tile([P, NPAIR, s], BF16)
            kT = singles.tile([P, NPAIR, s], BF16)
            v_sb = singles.tile([P, s // P, hpc, DH + 1], BF16)
            aTn = singles.tile([P, NPAIR, s], BF16)

            # ---- constant / weight loads ----
            nc.sync.dma_start(gm0_sb[:], gm0_dram[:])
            nc.sync.dma_start(gm1_sb[:], gm1_dram[:])
            nc.sync.dma_start(out=wqk_sb[:],
                              in_=wqk.rearrange("(c p) n -> p c n", p=P))
            nc.sync.dma_start(out=wv_sb[:],
                              in_=wv.rearrange("(c p) n -> p c n", p=P))
            nc.sync.dma_start(out=wp_sb[:],
                              in_=wp.rearrange("(c p) n -> p c n", p=P))
            nc.sync.dma_start(out=bqk_sb[:],
                              in_=bqk.rearrange("(ct p) -> p ct", p=P))
            nc.sync.dma_start(out=bv_rep[:], in_=bv.rearrange("(a b) -> a b", a=1).to_broadcast((P, VC)))
            nc.sync.dma_start(out=bp_rep[:], in_=bph.rearrange("(a b) -> a b", a=1).to_broadcast((P, d)))
            # ones column of v' (fused row-sum trick)
            nc.vector.memset(v_sb[:, :, :, DH], 1.0)

            for rep in range(reps):
              XB = 2 if NQT % 2 == 0 else 1
              xts2 = {}
              for tt in range(NQT):
                  ts0 = tt * QTS
                  # ---- x^T tiles (feature-major); XB token tiles per DMA ----
                  if tt % XB == 0:
                      xts2 = {}
                      for kc in range(KC):
                          xt2 = xt_pool.tile([P, XB * QTS], BF16)
                          nc.sync.dma_start(
                              out=xt2[:],
                              in_=xb[ts0:ts0 + XB * QTS,
                                     kc * P:(kc + 1) * P],
                              transpose=True)
                          xts2[kc] = xt2
                  off = (tt % XB) * QTS
                  xts = [xts2[kc][:, off:off + QTS] for kc in range(KC)]

                  # ---- q^T / k^T projection (feature-major out) ----
                  for ct in range(2 * VC // P):
                      ps = mm_ps.tile([P, QTS], FP32, tag="mm512")
                      for kc in range(KC):
                          nc.tensor.matmul(
                              ps[:], wqk_sb[:, kc, ct * P:(ct + 1) * P],
                              xts[kc][:], start=(kc == 0), stop=(kc == KC - 1))
                      pair, is_k = ct % NPAIR, ct // NPAIR
                      dst = (kT if is_k else qT)[:, pair, ts0:ts0 + QTS]
                      nc.vector.tensor_scalar_add(dst, ps[:], bqk_sb[:, ct:ct + 1])

                  # ---- v projection (token-major out) ----
                  for sub in range(QTS // P):
                      ps = mm_ps.tile([P, VC], FP32, tag="mm512")
                      for kc in range(KC):
                          nc.tensor.matmul(
                              ps[:], xts[kc][:, sub * P:(sub + 1) * P],
                              wv_sb[:, kc, :], start=(kc == 0),
                              stop=(kc == KC - 1))
                      vt = tt * (QTS // P) + sub
                      nc.vector.tensor_add(
                          v_sb[:, vt, :, 0:DH],
                          ps[:].rearrange("p (h e) -> p h e", e=DH),
                          bv_rep[:].rearrange("p (h e) -> p h e", e=DH))

                  # ---- attention for query tile tt, all head pairs ----
                  j = tt
                  nkt = 4 * (j + 1)  # causal: k tiles 0 .. nkt-1
                  for pair in range(NPAIR):
                      at_A = at_ps.tile([P, QTS], FP32, tag="atA")
                      at_B = at_ps.tile([P, QTS], FP32, tag="atB")
                      for grp in range(nkt // 2):
                          sc = sc_ps.tile([P, 2048], FP32, tag="sc")
                          for i in range(2):
                              kt = 2 * grp + i
                              nc.tensor.matmul(
                                  sc[:, i * 512:(i + 1) * 512],
                                  kT[0:DH, pair, kt * P:(kt + 1) * P],
                                  qT[0:DH, pair, ts0:ts0 + QTS],
                                  start=True, stop=True)
                              nc.tensor.matmul(
                                  sc[:, 1024 + i * 512:1024 + (i + 1) * 512],
                                  kT[DH:P, pair, kt * P:(kt + 1) * P],
                                  qT[DH:P, pair, ts0:ts0 + QTS],
                                  start=True, stop=True)
                          pr = probs_pool.tile([P, 2048], BF16)
                          nc.scalar.activation(pr[:], sc[:], AF.Exp,
                                               scale=1.0 / np.sqrt(DH))
                          # causal mask: one mul with precomputed group mask
                          if grp == 2 * j:
                              nc.vector.tensor_mul(pr[:], pr[:], gm0_sb[:])
                          elif grp == 2 * j + 1:
                              nc.vector.tensor_mul(pr[:], pr[:], gm1_sb[:])
                          for i in range(2):
                              kt = 2 * grp + i
                              for h01, at in ((0, at_A), (1, at_B)):
                                  nc.tensor.matmul(
                                      at[0:DH + 1, :],
                                      v_sb[:, kt, 2 * pair + h01, :],
                                      pr[:, h01 * 1024 + i * 512:
                                         h01 * 1024 + (i + 1) * 512],
                                      start=(kt == 0), stop=(kt == nkt - 1))
                      # ---- normalize: a^T / rowsum, store to aTn ----
                      # batch both heads' recip-sums into one staging DMA +
                      # one broadcast DMA per (pair, qtile)
                      rs = norm_pool.tile([1, 2 * QTS], FP32, tag="rs")
                      nc.vector.reciprocal(rs[:, 0:QTS], at_A[DH:DH + 1, :])
                      nc.vector.reciprocal(rs[:, QTS:], at_B[DH:DH + 1, :])
                      nc.sync.dma_start(out=rsd[2 * pair:2 * pair + 1, :],
                                        in_=rs[:])
                      rc = norm_pool.tile([P, 2 * QTS], FP32, tag="rc")
                      nc.sync.dma_start(
                          out=rc[:],
                          in_=rsd[2 * pair:2 * pair + 1, :]
                          .to_broadcast((P, 2 * QTS)))
                      nc.vector.tensor_mul(
                          aTn[0:DH, pair, ts0:ts0 + QTS],
                          at_A[0:DH, :], rc[0:DH, 0:QTS])
                      tmp = norm_pool.tile([DH, QTS], BF16, tag="tmpB")
                      nc.vector.tensor_mul(tmp[:], at_B[0:DH, :],
                                           rc[0:DH, QTS:])
                      nc.sync.dma_start(
                          out=aTn[DH:P, pair, ts0:ts0 + QTS],
                          in_=tmp[:])

                  # ---- partial out-projection for this token tile ----
                  for sub in range(QTS // P):
                      ysb = y_pool.tile([P, d], FP32)
                      t0 = ts0 + sub * P
                      for n in range(NOUT):
                          ps = mm_ps.tile([P, 512], FP32, tag="mm512")
                          for pc in range(PC):
                              nc.tensor.matmul(
                                  ps[:], aTn[:, pc, t0:t0 + P],
                                  wp_sb[:, pc, n * 512:(n + 1) * 512],
                                  start=(pc == 0), stop=(pc == PC - 1))
                          nc.vector.tensor_add(ysb[:, n * 512:(n + 1) * 512],
                                               ps[:],
                                               bp_rep[:, n * 512:(n + 1) * 512])
                      if "yout" not in ablate:
                        nc.sync.dma_start(out=yp[t0:t0 + P, :], in_=ysb[:])

            if dbg:
                for name, src, dst in (("qT", qT, qTo), ("kT", kT, kTo),
                                       ("v", v_sb, vo), ("aTn", aTn, aTno)):
                    t = singles.tile(list(src.shape), FP32, tag="d" + name)
                    nc.vector.tensor_copy(t[:], src[:])
                    nc.sync.dma_start(out=dst[:], in_=t[:])

    nc.compile()
    return nc


_NC_CACHE = {}


def _get_nc():
    if "nc" not in _NC_CACHE:
        _NC_CACHE["nc"] = build_nc()
    return _NC_CACHE["nc"]


def make_in_maps(x, w_attn, b_attn, w_proj, b_proj):
    """Host-side sharding: batch c//2, head-group c%2."""
    VC = HPC * DH  # 512
    wq, wk, wv = w_attn[:, :D], w_attn[:, D:2 * D], w_attn[:, 2 * D:]
    bq, bk, bv = b_attn[:D], b_attn[D:2 * D], b_attn[2 * D:]
    in_maps = []
    for c in range(N_CORES):
        b, g = c // 2, c % 2
        sl = slice(g * VC, (g + 1) * VC)
        bf = ml_dtypes.bfloat16
        in_maps.append({
            "xb": np.ascontiguousarray(x[b].astype(bf)),
            "wqk": np.ascontiguousarray(
                np.concatenate([wq[:, sl], wk[:, sl]], axis=1).astype(bf)),
            "wv": np.ascontiguousarray(wv[:, sl].astype(bf)),
            "wp": np.ascontiguousarray(
                w_proj[g * VC:(g + 1) * VC, :].astype(bf)),
            "bqk": np.ascontiguousarray(
                np.concatenate([bq[sl], bk[sl]])),
            "bv": np.ascontiguousarray(bv[sl]),
            "bph": np.ascontiguousarray(b_proj * 0.5),
        })
    return in_maps


def kernel(x, w_attn, b_attn, w_proj, b_proj):
    x = np.asarray(x, dtype=np.float32)
    w_attn = np.asarray(w_attn, dtype=np.float32)
    b_attn = np.asarray(b_attn, dtype=np.float32)
    w_proj = np.asarray(w_proj, dtype=np.float32)
    b_proj = np.asarray(b_proj, dtype=np.float32)

    nc = _get_nc()
    in_maps = make_in_maps(x, w_attn, b_attn, w_proj, b_proj)
    res = run_bass_kernel_spmd(nc, in_maps, core_ids=list(range(N_CORES)))
    out = np.empty((B, S, D), dtype=np.float32)
    for b in range(B):
        out[b] = res.results[2 * b]["yp"] + res.results[2 * b + 1]["yp"]
    return out

